# revision 2
# baseline (speedup 1.0000x reference)
"""Trainium2 Bass kernel for nn_CustomLinearLayer:
    out = input @ (S * THETA).T + bias
with input [4096, 2048] f32, S/THETA [512, 2048] f32, bias [512] f32.

Strategy: data-parallel shard of the batch across 8 NeuronCores
(512 rows each); S/THETA/bias replicated. Per core:
  - DMA X shard, S, THETA in natural [rows, K] layout
  - W = S * THETA elementwise on VectorE
  - transpose X and W into [K, rows] tiles with TensorE (identity matmul,
    exact in fp32), PSUM->SBUF copybacks split across VectorE/ScalarE
  - out.T[o, b] = sum_k W.T[k, o-slice].T @ X.T[k, :] accumulated in PSUM,
    matmul in float32r (fp32 bits on the fast PE path)
  - bias added during the PSUM->SBUF copyback (per-partition scalar add)
  - DMA out.T [512, 512] per core; host glue transposes/concats shards.
"""

import numpy as np

N_CORES = 8
BATCH, OUT_DIM, IN_DIM = 4096, 512, 2048
B_CORE = BATCH // N_CORES  # 512 batch rows per core
P = 128
KT = IN_DIM // P  # 16 k-tiles
BT = B_CORE // P  # 4 batch subtiles
OT = OUT_DIM // P  # 4 output subtiles

# matmul dtype mode: "f32r" (fp32 bits, fast PE path), "f32" (4 cyc/row),
# "bf16" (inputs rounded to bf16, fp32 accumulate)
MM_MODE = "f32r"

_CACHE = {}


def _build(mode):
    from contextlib import ExitStack

    import concourse.bass as bass
    import concourse.tile as tile
    from concourse import bacc, mybir
    from concourse.masks import make_identity

    f32 = mybir.dt.float32
    f32r = mybir.dt.float32r
    bf16 = mybir.dt.bfloat16

    nc = bacc.Bacc("TRN2", target_bir_lowering=False, debug=False,
                   num_devices=N_CORES)

    x_d = nc.dram_tensor("x", [B_CORE, IN_DIM], f32, kind="ExternalInput").ap()
    s_d = nc.dram_tensor("s", [OUT_DIM, IN_DIM], f32, kind="ExternalInput").ap()
    th_d = nc.dram_tensor("th", [OUT_DIM, IN_DIM], f32, kind="ExternalInput").ap()
    # bias pre-arranged on host as [128, OT]: b[p, m] = bias[m*128 + p]
    b_d = nc.dram_tensor("b", [P, OT], f32, kind="ExternalInput").ap()
    # out.T layout: [OUT_DIM, B_CORE]
    o_d = nc.dram_tensor("o", [OUT_DIM, B_CORE], f32, kind="ExternalOutput").ap()

    # transpose copyback dst dtype == matmul operand dtype; the copyback
    # engine rounds fp32 PSUM into this format (required for f32r)
    tr_dtype = {"f32r": f32r, "bf16": bf16, "f32": f32}[mode]

    def mm_ap(ap):
        return ap

    with tile.TileContext(nc) as tc, ExitStack() as ctx:
        const = ctx.enter_context(tc.tile_pool(name="const", bufs=1))
        identity = const.tile([P, P], f32)
        make_identity(nc, identity[:])
        bias_col = const.tile([P, OT], f32)
        nc.sync.dma_start(bias_col[:], b_d[:])

        x_pool = ctx.enter_context(tc.tile_pool(name="x", bufs=2))
        s_pool = ctx.enter_context(tc.tile_pool(name="s", bufs=2))
        th_pool = ctx.enter_context(tc.tile_pool(name="th", bufs=2))
        w_pool = ctx.enter_context(tc.tile_pool(name="w", bufs=2))
        big = ctx.enter_context(tc.tile_pool(name="big", bufs=1))
        out_pool = ctx.enter_context(tc.tile_pool(name="out", bufs=2))
        tr_psum = ctx.enter_context(
            tc.tile_pool(name="trps", bufs=6, space="PSUM"))
        mm_psum = ctx.enter_context(
            tc.tile_pool(name="mmps", bufs=2, space="PSUM"))

        # transposed operands, resident: [k-part, k-tile, row]
        xt = big.tile([P, KT, B_CORE], tr_dtype)
        wt = big.tile([P, KT, OUT_DIM], tr_dtype)

        ncopy = 0

        def copyback(dst, src):
            # alternate PSUM->SBUF copyback between VectorE and ScalarE
            nonlocal ncopy
            if ncopy % 2 == 0:
                nc.vector.tensor_copy(dst, src)
            else:
                nc.scalar.copy(dst, src)
            ncopy += 1

        # X path: load natural b-tiles, transpose each k-chunk on PE
        for bt in range(BT):
            x_t = x_pool.tile([P, IN_DIM], f32)
            nc.sync.dma_start(x_t[:], x_d[bt * P:(bt + 1) * P, :])
            for k in range(KT):
                pt = tr_psum.tile([P, P], f32)
                nc.tensor.transpose(pt[:], x_t[:, k * P:(k + 1) * P], identity[:])
                copyback(xt[:, k, bt * P:(bt + 1) * P], pt[:])

        # W path: load S/THETA o-tiles, multiply, transpose each k-chunk
        for m in range(OT):
            s_t = s_pool.tile([P, IN_DIM], f32)
            nc.sync.dma_start(s_t[:], s_d[m * P:(m + 1) * P, :])
            th_t = th_pool.tile([P, IN_DIM], f32)
            nc.sync.dma_start(th_t[:], th_d[m * P:(m + 1) * P, :])
            w_t = w_pool.tile([P, IN_DIM], f32)
            nc.vector.tensor_mul(w_t[:], s_t[:], th_t[:])
            for k in range(KT):
                pt = tr_psum.tile([P, P], f32)
                nc.tensor.transpose(pt[:], w_t[:, k * P:(k + 1) * P], identity[:])
                copyback(wt[:, k, m * P:(m + 1) * P], pt[:])

        # MM: out.T[o-slice, :] = sum_k wt[:, k, o-slice].T @ xt[:, k, :]
        for m in range(OT):
            ps = mm_psum.tile([P, B_CORE], f32)
            for k in range(KT):
                nc.tensor.matmul(
                    ps[:],
                    mm_ap(wt[:, k, m * P:(m + 1) * P]),
                    mm_ap(xt[:, k, :]),
                    start=(k == 0),
                    stop=(k == KT - 1),
                )
            o_t = out_pool.tile([P, B_CORE], f32)
            # fused bias add: out.T[o, b] = psum[o, b] + bias[o]
            nc.vector.tensor_scalar_add(o_t[:], ps[:], bias_col[:, m:m + 1])
            nc.sync.dma_start(o_d[m * P:(m + 1) * P, :], o_t[:])

    nc.compile()
    return nc


def kernel(input, S, THETA, bias):
    from concourse.bass_utils import run_bass_kernel_spmd

    if MM_MODE not in _CACHE:
        _CACHE[MM_MODE] = _build(MM_MODE)
    nc = _CACHE[MM_MODE]

    input = np.ascontiguousarray(input, dtype=np.float32)
    S = np.ascontiguousarray(S, dtype=np.float32)
    THETA = np.ascontiguousarray(THETA, dtype=np.float32)
    bias = np.ascontiguousarray(bias, dtype=np.float32)
    b_host = np.ascontiguousarray(bias.reshape(OT, P).T)  # [128, OT]

    in_maps = [
        {
            "x": np.ascontiguousarray(input[c * B_CORE:(c + 1) * B_CORE]),
            "s": S,
            "th": THETA,
            "b": b_host,
        }
        for c in range(N_CORES)
    ]
    res = run_bass_kernel_spmd(nc, in_maps, core_ids=list(range(N_CORES)))
    out = np.empty((BATCH, OUT_DIM), dtype=np.float32)
    for c in range(N_CORES):
        out[c * B_CORE:(c + 1) * B_CORE, :] = res.results[c]["o"].T
    return out
